# revision 25
# baseline (speedup 1.0000x reference)
"""Distributed Trainium2 Bass kernel for pre-LN multi-head attention (v3).

Reference computation (per batch b of 2, seq n=2048, dim=1024, 16 heads x 64):
    xn = LayerNorm(x) * gamma + beta
    q, k = split(xn @ W_qk); v = xn @ W_v
    out = softmax(q k^T / 8) v  (per head)
    y = out @ W_out + b_out

Sharding (head-parallel attention): 8 cores = 2 batch groups x 4 head groups.
Core i owns batch g=i//4 and heads [4r, 4r+4) with r=i%4. Each core receives
the FULL batch x (host-cast bf16, token-rotated, see below), runs LayerNorm
over all 2048 tokens (4x redundant but cheap, hides under DMA), computes
K^T/V/Q projections for its 4 heads over all tokens (same PE cycles as a
token-sharded projection), and runs attention for its 4 heads over all 2048
queries x 2048 keys. No mid-kernel K/V AllGather: that collective stream only
starts after the ~55us CC mesh bring-up and runs at ~75GB/s, costing ~50us of
PE idle in the token-sharded design.

The ACT engine's exp stream is the binding resource of the attention phase
(16.8M exps at 1 elem/cycle/lane ~= 140us); everything is organized to start
it as early as possible and keep it dense:
- proj emission order K, V, Q(heads 0-1), attention units (h0, h1) of the
  first quarter, Q(heads 2-3), rest — so the first S^T/exp work reaches the
  PE/ACT queues ~40us before the projections fully drain.
- PSUM pools are shared between proj chains and attention (pp + ps_s halves)
  so attention pools coexist with proj pools without exceeding 8 banks.

Communication is a late attention-output exchange, after the CC mesh is up.
The host rotates each core's tokens by (r+1)*512 so program position-quarter
d = absolute quarter (r+1+d)%4 and position 3 is the core's OWN quarter:
foreign quarters finish first and AllGather (one 256KB-in op per quarter)
within the batch group mid-attention, fully hidden; the own quarter needs no
exchange. Each core extracts, from gather d, the tile of the one peer whose
position-d is this core's absolute quarter — a single DMA whose source offset
is computed at runtime from partition_id() (rank-dependent extraction in a
single SPMD program). Tiles land in att_full in relative-slot order; W_out
rows are host-rotated to match, so the output projection (full 16-head
contraction for the own 512-token quarter) is entirely static. y^T is written
directly; the host assembles 8 [1024, 512] shards, no reduction.
"""
import sys
import types

sys.path.insert(0, "/opt/trn_rl_repo")

# Register the NTFF profile hook that trn_boot skips when the image's antenv
# lacks axon_hooks, so run_bass_kernel_spmd(trace=True) can report exec time.
if "antenv.axon_hooks" not in sys.modules:
    try:
        from trn_agent_boot.trn_boot import _ntff_profile_via_ctypes

        _hook = _ntff_profile_via_ctypes("/opt/axon/libaxon_pjrt.so")
    except Exception:
        _hook = None
    _mod = types.ModuleType("antenv.axon_hooks")
    _mod.get_axon_ntff_profile_hook = lambda: _hook
    _mod.set_axon_ntff_profile_hook = lambda h: None
    sys.modules["antenv.axon_hooks"] = _mod

from contextlib import ExitStack

import ml_dtypes
import numpy as np
import concourse.bass as bass
import concourse.tile as tile
from concourse import bacc, mybir
from concourse.bass_utils import run_bass_kernel_spmd
from concourse.masks import make_identity

F32 = mybir.dt.float32
BF16 = mybir.dt.bfloat16
AF = mybir.ActivationFunctionType
ALU = mybir.AluOpType

B, N, DIM = 2, 2048, 1024
HEADS, DH = 16, 64
INNER = HEADS * DH  # 1024
SCALE = DH**-0.5
EPS = 1e-5

NCORES = 8
GROUP = 4          # cores per batch group (head groups / output quarters)
LH = HEADS // GROUP  # 4 local heads per core
LIN = LH * DH      # 256 local inner dims
NQ = N // GROUP    # 512 tokens per output quarter
DC = DIM // 128    # 8 dim chunks
KCH = N // 128     # 16 key chunks of 128 tokens
KPAIR = KCH // 2   # exp batches of 2 key chunks
TG = 4             # token groups of 512 for LN/proj pipeline

MMDT = BF16        # matmul operand storage dtype

REPLICA_GROUPS = [[0, 1, 2, 3], [4, 5, 6, 7]]

VSTR = 448         # per-key-chunk vones stride: 2x[ones64|V_2c|V_2c+1] + ones64
# DVE "exp": u = round(a*logit + (127*128 - c)); the int16 bit pattern of u
# IS bf16(exp(logit)) up to the linear-interp-between-octaves error (~1.5%
# rms, validated end-to-end at rel_err 1.14e-2 vs the 2e-2 gate when applied
# to half the key chunks). psum holds 2*S so a absorbs SCALE/2.
SCH_A = (128.0 / float(np.log(2.0))) * (SCALE / 2.0)
SCH_B = 127.0 * 128.0 - 8.0
QSZ = 2 * 128 * NQ  # one staged quarter tile (both head pairs), flat


def build_nc():
    nc = bacc.Bacc(num_devices=NCORES)

    x = nc.dram_tensor("x", [N, DIM], MMDT, kind="ExternalInput")
    gbb = nc.dram_tensor("gbb", [128, 24], F32, kind="ExternalInput")
    wqk_k = nc.dram_tensor("wqk_k", [DIM, LIN], MMDT, kind="ExternalInput")
    wqk_q = nc.dram_tensor("wqk_q", [DIM, LIN], MMDT, kind="ExternalInput")
    w_v = nc.dram_tensor("W_v", [DIM, LIN], MMDT, kind="ExternalInput")
    w_out = nc.dram_tensor("W_out", [INNER, DIM], MMDT, kind="ExternalInput")
    out = nc.dram_tensor("out", [DIM, NQ], MMDT, kind="ExternalOutput")

    with tile.TileContext(nc) as tc, ExitStack() as ctx:
        pool = lambda name, bufs, **kw: ctx.enter_context(
            tc.tile_pool(name=name, bufs=bufs, **kw)
        )
        consts = pool("consts", 1)
        dram = pool("dram", 1, space="DRAM")
        kv = pool("kv", 1)          # kt_sb + vones + q_t (live whole kernel)
        att_pool = pool("att", 1)
        wo_pool = pool("wo", 1)
        small = pool("small", 8)
        es_pool = pool("es", 8)
        es16_pool = pool("es16", 6)
        rp_pool = pool("rp", 2)
        y_pool = pool("y", 2)
        pp = pool("pp", 2, space="PSUM")      # acc chains / AV accumulators

        # ---- constants ---------------------------------------------------
        gbb_t = consts.tile([128, 24], F32)   # [gamma | beta | b_out] per c
        nc.sync.dma_start(gbb_t[:], gbb[:, :])
        ident = consts.tile([128, 128], MMDT)
        make_identity(nc, ident[:])
        eps_sb = consts.tile([128, 1], F32)
        nc.vector.memset(eps_sb[:], EPS)
        # PE warmup: ramp the clock while input DMAs are in flight.
        wps = pp.tile([128, 512], F32, tag="acc", name="warmup")
        for i in range(24):
            nc.tensor.matmul(
                wps[:, 0:128], ident[:], ident[:], start=(i == 0), stop=(i == 23)
            )

        cc_ins = [dram.tile([QSZ], MMDT, name=f"cc_in{d}") for d in range(3)]
        cc_outs = [
            dram.tile([GROUP * QSZ], MMDT, name=f"cc_out{d}") for d in range(3)
        ]

        # K^T per head, duplicated across both 64-row halves so the S^T
        # matmuls contract over K=128 (computing 2*S, folded into the exp
        # scale; K=64 matmuls were observed to hold the HAM clock at 1.2GHz).
        kt_sb = kv.tile([128, LH * N], MMDT)
        # V interleaved with ones blocks: key chunk kc spans [kc*448, +448):
        # [ones64 | V_h0 | V_h1 | ones64 | V_h2 | V_h3 | ones64]. Head h's AV
        # lhsT = cols kc*448 + (h//2)*192 + (h%2)*128, len 128: even heads
        # [ones | V] (AV rows 0:64 = colsum, 64:128 = data), odd swapped.
        vones = kv.tile([128, KCH * VSTR], MMDT)
        # Q^T duplicated per head like K^T (see kt_sb note).
        q_t = kv.tile([128, LH * N], MMDT)
        # attention output^T: position-quarter d, head pair hc at chunk
        # (d*2 + hc), [128, 512] with head parity on the 64-row halves.
        att_t = att_pool.tile([128, GROUP * 2 * NQ], MMDT)
        wout_sb = wo_pool.tile([128, DC * DIM], MMDT)
        att_full = wo_pool.tile([128, DC * NQ], MMDT)

        for kc in range(KCH):
            ones_base = vones[:, kc * VSTR : kc * VSTR + 64]
            nc.vector.memset(
                bass.AP(
                    tensor=ones_base.tensor,
                    offset=ones_base.offset,
                    ap=[ones_base.ap[0], [192, 3], [1, 64]],
                ),
                1.0,
            )

        # ---- attention helpers (outer pools only) ------------------------
        def av_lhs(h, kc):
            base = kc * VSTR + (h // 2) * 192 + (h % 2) * 128
            return vones[:, base : base + 128]

        def head_divide(h, d, po):
            hp = (h % 2) * 64
            hc = h // 2
            cb, dp = hp, 64 - hp
            recip = rp_pool.tile([128, 1024], F32, tag="recip", name="recip")
            nc.vector.tensor_copy(recip[0:64, 512:1024], po[cb : cb + 64, :])
            nc.vector.reciprocal_approx_fast(
                recip[0:64, 0:512], recip[0:64, 512:1024]
            )
            nc.vector.tensor_mul(
                att_t[hp : hp + 64, (d * 2 + hc) * NQ : (d * 2 + hc + 1) * NQ],
                po[dp : dp + 64, :],
                recip[0:64, 0:512],
            )

        def head_unit(ps_pool, h, d):
            # interleaved S/exp/AV pipeline for one (head, position-quarter);
            # exp alternates between ACT (exact) and DVE (Schraudolph bf16)
            # so neither engine is the wall
            po = pp.tile([128, 512], F32, tag="acc", name="po")
            for pr in range(KPAIR):
                pss = ps_pool.tile([128, 1024], F32, tag="pss", name="pss")
                for j in range(2):
                    kc = 2 * pr + j
                    nc.tensor.matmul(
                        pss[:, j * 512 : (j + 1) * 512],
                        kt_sb[:, h * N + kc * 128 : h * N + (kc + 1) * 128],
                        q_t[:, h * N + d * 512 : h * N + (d + 1) * 512],
                        start=True,
                        stop=True,
                    )
                if pr % 2 == 0:
                    es = es_pool.tile([128, 1024], MMDT, tag="es", name="es")
                    # psum holds 2*S (duplicated operands) -> halve the scale
                    nc.scalar.activation(es[:], pss[:], AF.Exp, scale=SCALE / 2)
                    esv = es[:]
                else:
                    e16 = es16_pool.tile(
                        [128, 1024], mybir.dt.int16, tag="e16", name="e16"
                    )
                    nc.vector.tensor_scalar(
                        out=e16[:],
                        in0=pss[:],
                        scalar1=SCH_A,
                        scalar2=SCH_B,
                        op0=ALU.mult,
                        op1=ALU.add,
                    )
                    esv = e16[:].bitcast(MMDT)
                for j in range(2):
                    kc = 2 * pr + j
                    nc.tensor.matmul(
                        po[:],
                        av_lhs(h, kc),
                        esv[:, j * 512 : (j + 1) * 512],
                        start=(pr == 0 and j == 0),
                        stop=(pr == KPAIR - 1 and j == 1),
                    )
            head_divide(h, d, po)

        sp = nc.engines[mybir.EngineType.SP]

        def stage_and_gather(d):
            # stage position-quarter d's full tile (both head pairs), then
            # AllGather it within the batch group
            nc.sync.dma_start(
                cc_ins[d][:].rearrange("(p f) -> p f", f=2 * NQ),
                att_t[:, (d * 2) * NQ : (d * 2 + 2) * NQ],
            )
            nc.gpsimd.collective_compute(
                "AllGather",
                ALU.bypass,
                replica_groups=REPLICA_GROUPS,
                ins=[cc_ins[d][:].opt()],
                outs=[cc_outs[d][:].opt()],
            )

        def extract(d, rid):
            # gather d carries every rank's position-d tile; MY quarter's
            # tile in it comes from rank (rid-1-d)%4 (the peer whose
            # position-d is my absolute quarter). It lands in att_full
            # relative slot d+1 (static) -- W_out rows are host-rotated to
            # the relative-slot order.
            roff = sp.compute_val(((rid + 3 - d) % GROUP) * QSZ)
            blk = cc_outs[d][bass.ds(roff, QSZ)]
            src = bass.AP(
                tensor=blk.tensor,
                offset=blk.offset,
                ap=[[2 * NQ, 128], [1, 2 * NQ]],
            )
            nc.sync.dma_start(
                att_full[:, (d + 1) * 2 * NQ : (d + 2) * 2 * NQ], src
            )

        def emit_wout(cs):
            for c in cs:
                nc.sync.dma_start(
                    wout_sb[:, c * DIM : (c + 1) * DIM],
                    w_out[c * 128 : (c + 1) * 128, :],
                )

        # ---- LN + projections --------------------------------------------
        with ExitStack() as proj_ctx:
            ppool = lambda name, bufs, **kw: proj_ctx.enter_context(
                tc.tile_pool(name=name, bufs=bufs, **kw)
            )
            ptr = ppool("ptr", 2, space="PSUM")  # transpose targets
            pkv = ppool("pkv", 2, space="PSUM")  # second accumulation chain
            xw = ppool("xw", 1)
            x_pool = ppool("xp", 8)
            xn_pool = ppool("xn", 2)
            vt_pool = ppool("vt", 2)
            xnt = xw.tile([128, DC * N], MMDT)
            wk_sb = xw.tile([128, DC * LIN], MMDT)
            wv_sb = xw.tile([128, DC * LIN], MMDT)
            wq_sb = xw.tile([128, DC * LIN], MMDT)

            # x chunk tiles from a rotating pool; DMAs emitted in
            # consumption order interleaved with the weight loads,
            # <=256KB per DMA.
            x_tiles = []

            def emit_x(ts):
                for t in ts:
                    xt = x_pool.tile([128, DIM], MMDT, tag="x", name=f"x{t}")
                    nc.sync.dma_start(xt[:], x[t * 128 : (t + 1) * 128, :])
                    x_tiles.append(xt)

            emit_x(range(0, 8))
            for w_sb, w_hbm in ((wk_sb, wqk_k), (wv_sb, w_v), (wq_sb, wqk_q)):
                for c in range(DC):
                    nc.sync.dma_start(
                        w_sb[:, c * LIN : (c + 1) * LIN],
                        w_hbm[c * 128 : (c + 1) * 128, :],
                    )
            emit_x(range(8, 16))

            def ln_group(tg):
                xn_t = xn_pool.tile([128, 4 * DIM], MMDT, tag="xn")
                for i in range(4):
                    xt = x_tiles[tg * 4 + i][:]
                    xg = xt.rearrange("p (n s) -> p n s", s=512)
                    stats = small.tile([128, 2, 6], F32)
                    for sgi in range(2):
                        nc.vector.bn_stats(stats[:, sgi, :], xg[:, sgi, :])
                    mv = small.tile([128, 2], F32)
                    nc.vector.bn_aggr(mv[:], stats[:])
                    rstd = small.tile([128, 1], F32)
                    nc.scalar.activation(rstd[:], mv[:, 1:2], AF.Sqrt, bias=eps_sb[:])
                    nc.vector.reciprocal(rstd[:], rstd[:])
                    nc.vector.tensor_scalar(
                        out=xn_t[:, i * DIM : (i + 1) * DIM],
                        in0=xt,
                        scalar1=mv[:, 0:1],
                        scalar2=rstd[:],
                        op0=ALU.subtract,
                        op1=ALU.mult,
                    )
                # transpose to [dim, tokens], fusing gamma/beta; split the
                # scale/cast between Scalar and Vector engines
                for c in range(DC):
                    pt = ptr.tile([128, 512], MMDT, tag="tr")
                    for i in range(4):
                        nc.tensor.transpose(
                            pt[:, i * 128 : (i + 1) * 128],
                            xn_t[:, i * DIM + c * 128 : i * DIM + (c + 1) * 128],
                            ident[:],
                        )
                    dst = xnt[:, c * N + tg * 512 : c * N + (tg + 1) * 512]
                    if (c + tg) % 2 == 0:
                        nc.scalar.activation(
                            dst,
                            pt[:],
                            AF.Identity,
                            bias=gbb_t[:, 8 + c : 9 + c],
                            scale=gbb_t[:, c : c + 1],
                        )
                    else:
                        nc.vector.tensor_scalar(
                            out=dst,
                            in0=pt[:],
                            scalar1=gbb_t[:, c : c + 1],
                            scalar2=gbb_t[:, 8 + c : 9 + c],
                            op0=ALU.mult,
                            op1=ALU.add,
                        )

            def two_chains(width=512):
                c0 = pp.tile([128, 512], F32, tag="acc", name="c0")
                c1 = pkv.tile([128, 512], F32, tag="kvacc", name="c1")
                return [c0[:, 0:width], c1[:, 0:width]]

            def k_proj(s):
                pqs = two_chains()
                for c in range(DC):
                    for m in range(2):
                        nc.tensor.matmul(
                            pqs[m],
                            wk_sb[:, c * LIN + m * 128 : c * LIN + (m + 1) * 128],
                            xnt[:, c * N + s * 512 : c * N + (s + 1) * 512],
                            start=(c == 0),
                            stop=(c == DC - 1),
                        )
                for m in range(2):
                    for lh in range(2):
                        h = 2 * m + lh
                        span = slice(h * N + s * 512, h * N + (s + 1) * 512)
                        src = pqs[m][lh * 64 : lh * 64 + 64, :]
                        if (m + lh) % 2 == 0:
                            nc.vector.tensor_copy(kt_sb[0:64, span], src)
                        else:
                            nc.scalar.copy(kt_sb[0:64, span], src)
                        # K=128 duplication: SBUF->SBUF DMA, no HBM traffic
                        nc.sync.dma_start(kt_sb[64:128, span], kt_sb[0:64, span])

            def v_proj(s):
                # v^T chains [128 vdims (head pair m), 512 tok] with 512-col
                # moving (2x the per-matmul efficiency of a 256-col [tok,
                # vdim] projection), then PE-transpose into vones' [token,
                # vdim] blocks
                pvs = two_chains()
                for c in range(DC):
                    for m in range(2):
                        nc.tensor.matmul(
                            pvs[m],
                            wv_sb[:, c * LIN + m * 128 : c * LIN + (m + 1) * 128],
                            xnt[:, c * N + s * 512 : c * N + (s + 1) * 512],
                            start=(c == 0),
                            stop=(c == DC - 1),
                        )
                for m in range(2):
                    vt = vt_pool.tile([128, 512], MMDT, tag="vt")
                    if m == 0:
                        nc.vector.tensor_copy(vt[:], pvs[m])
                    else:
                        nc.scalar.copy(vt[:], pvs[m])
                    pt = ptr.tile([128, 512], MMDT, tag="tr")
                    for tc in range(4):
                        nc.tensor.transpose(
                            pt[:, tc * 128 : (tc + 1) * 128],
                            vt[:, tc * 128 : (tc + 1) * 128],
                            ident[:],
                        )
                    vdst = vones[:, (s * 4) * VSTR + 64 + m * 192 :]
                    dst_ap = bass.AP(
                        tensor=vdst.tensor,
                        offset=vdst.offset,
                        ap=[vdst.ap[0], [VSTR, 4], [1, 128]],
                    )
                    src_ap = pt[:].rearrange("p (n f) -> p n f", f=128)
                    if m == 0:
                        nc.scalar.copy(dst_ap, src_ap)
                    else:
                        nc.vector.tensor_copy(dst_ap, src_ap)

            def q_proj_m(m):
                # head pair m over all 4 spans: 4 interleaved chains
                c0 = pp.tile([128, 512], F32, tag="acc", name="q0")
                c1 = pp.tile([128, 512], F32, tag="acc", name="q1")
                c2 = pkv.tile([128, 512], F32, tag="kvacc", name="q2")
                c3 = pkv.tile([128, 512], F32, tag="kvacc", name="q3")
                pqs = [c0[:], c1[:], c2[:], c3[:]]
                for c in range(DC):
                    for s in range(TG):
                        nc.tensor.matmul(
                            pqs[s],
                            wq_sb[:, c * LIN + m * 128 : c * LIN + (m + 1) * 128],
                            xnt[:, c * N + s * 512 : c * N + (s + 1) * 512],
                            start=(c == 0),
                            stop=(c == DC - 1),
                        )
                for s in range(TG):
                    for lh in range(2):
                        h = 2 * m + lh
                        span = slice(h * N + s * 512, h * N + (s + 1) * 512)
                        src = pqs[s][lh * 64 : lh * 64 + 64, :]
                        if (s + lh) % 2 == 0:
                            nc.vector.tensor_copy(q_t[0:64, span], src)
                        else:
                            nc.scalar.copy(q_t[0:64, span], src)
                        nc.sync.dma_start(q_t[64:128, span], q_t[0:64, span])

            for tg in range(TG):
                ln_group(tg)
            for s in range(TG):
                k_proj(s)
            for s in range(TG):
                v_proj(s)
            def early_unit(h, d):
                # first-quarter unit emitted inside the proj scope, S^T in
                # [128, 512] singles from the pkv pool: the ACT exp stream
                # (the binding resource) starts ~40us before the remaining
                # projections drain
                po = pp.tile([128, 512], F32, tag="acc", name="po")
                for kc in range(KCH):
                    pss = pkv.tile([128, 512], F32, tag="kvacc", name="pse")
                    nc.tensor.matmul(
                        pss[:],
                        kt_sb[:, h * N + kc * 128 : h * N + (kc + 1) * 128],
                        q_t[:, h * N + d * 512 : h * N + (d + 1) * 512],
                        start=True,
                        stop=True,
                    )
                    es = es_pool.tile([128, 1024], MMDT, tag="es", name="es")
                    nc.scalar.activation(
                        es[:, 0:512], pss[:], AF.Exp, scale=SCALE / 2
                    )
                    nc.tensor.matmul(
                        po[:],
                        av_lhs(h, kc),
                        es[:, 0:512],
                        start=(kc == 0),
                        stop=(kc == KCH - 1),
                    )
                head_divide(h, d, po)

            # Q heads 0-1, then the first four attention units (heads 0-1
            # of positions 0 and 1 need only Q chunk m0); their exp backlog
            # covers the Qm1 projection with no ACT gap
            q_proj_m(0)
            early_unit(0, 0)
            early_unit(1, 0)
            q_proj_m(1)

        # ---- attention + exchange ----------------------------------------
        ps_s = ctx.enter_context(
            tc.tile_pool(name="ps_s", bufs=3, space="PSUM")
        )
        rid = sp.partition_id() % GROUP

        # quarter-major; position d = absolute quarter (rid+1+d)%4 via the
        # host token rotation, so position 3 is my OWN quarter and needs no
        # exchange. Foreign quarters gather mid-attention.
        for d in range(GROUP):
            for h in range(LH):
                if d == 0 and h < 2:
                    continue  # emitted inside proj_ctx above
                head_unit(ps_s, h, d)
            if d < 3:
                stage_and_gather(d)
                # spread the W_out loads (2MB total) across the quarter
                # boundaries; a single 8-chunk burst was observed to stretch
                # concurrent ACT exp instructions
                emit_wout(range(2 + d * 2, 4 + d * 2))
        emit_wout([0, 1])
        # own tile -> relative slot 0
        nc.sync.dma_start(att_full[:, 0 : 2 * NQ], att_t[:, 6 * NQ : 8 * NQ])
        for d in range(3):
            extract(d, rid)

        # ---- output projection y^T = W_out^T @ att^T + b_out -------------
        # four interleaved accumulation chains; foreign att chunks (landed
        # mid-attention) accumulate first, own chunks last
        C_ORDER = [2, 3, 4, 5, 6, 7, 0, 1]
        for mp in range(DC // 4):
            pys = []
            for mi in range(4):
                if mi < 2:
                    py = pp.tile([128, 512], F32, tag="acc", name=f"py{mi}")
                    pys.append(py[:])
                else:
                    py = ps_s.tile([128, 1024], F32, tag="pss", name=f"py{mi}")
                    pys.append(py[:, 0:512])
            for ci, c in enumerate(C_ORDER):
                for mi in range(4):
                    m = 4 * mp + mi
                    nc.tensor.matmul(
                        pys[mi],
                        wout_sb[:, c * DIM + m * 128 : c * DIM + (m + 1) * 128],
                        att_full[:, c * NQ : (c + 1) * NQ],
                        start=(ci == 0),
                        stop=(ci == DC - 1),
                    )
            for mi in range(4):
                m = 4 * mp + mi
                y_sb = y_pool.tile([128, 512], MMDT, tag="y")
                nc.vector.tensor_scalar(
                    out=y_sb[:],
                    in0=pys[mi],
                    scalar1=gbb_t[:, 16 + m : 17 + m],
                    scalar2=None,
                    op0=ALU.add,
                )
                nc.sync.dma_start(out[m * 128 : (m + 1) * 128, :], y_sb[:])

    nc.compile()
    return nc


_NC_CACHE = None


def _get_nc():
    global _NC_CACHE
    if _NC_CACHE is None:
        _NC_CACHE = build_nc()
    return _NC_CACHE


def _make_in_maps(x, ln_gamma, ln_beta, W_qk, W_v, W_out, b_out):
    mmnp = mybir.dt.np(MMDT)
    wqk = np.asarray(W_qk, dtype=np.float32)
    wv = np.asarray(W_v, dtype=np.float32)
    wo = np.asarray(W_out, dtype=np.float32)
    gamma = np.asarray(ln_gamma, dtype=np.float32).reshape(DC, 128).T
    beta = np.asarray(ln_beta, dtype=np.float32).reshape(DC, 128).T
    bout = np.asarray(b_out, dtype=np.float32).reshape(DC, 128).T
    gbb = np.ascontiguousarray(np.concatenate([gamma, beta, bout], axis=1))
    xf = np.asarray(x, dtype=np.float32)
    xb = [np.ascontiguousarray(xf[g]).astype(mmnp) for g in range(B)]
    in_maps = []
    for i in range(NCORES):
        g, r = i // GROUP, i % GROUP
        cols = slice(r * LIN, (r + 1) * LIN)
        kcols = slice(INNER + r * LIN, INNER + (r + 1) * LIN)
        # token rotation: program position-quarter d = absolute quarter
        # (r+1+d)%4, so position 3 is this core's own quarter
        xr = np.roll(xb[g], -(r + 1) * NQ, axis=0)
        # W_out rows in relative-slot order: slot s holds the heads of
        # rank (r-s)%4 (slot 0 = own heads)
        wor = np.concatenate(
            [
                wo[((r - s) % GROUP) * LIN : (((r - s) % GROUP) + 1) * LIN]
                for s in range(GROUP)
            ]
        )
        in_maps.append(
            {
                "x": np.ascontiguousarray(xr),
                "gbb": gbb,
                "wqk_k": np.ascontiguousarray(wqk[:, kcols]).astype(mmnp),
                "wqk_q": np.ascontiguousarray(wqk[:, cols]).astype(mmnp),
                "W_v": np.ascontiguousarray(wv[:, cols]).astype(mmnp),
                "W_out": np.ascontiguousarray(wor).astype(mmnp),
            }
        )
    return in_maps


def run(inputs: dict, trace: bool = False):
    """Run the distributed kernel; returns (full_output, BassKernelResults)."""
    nc = _get_nc()
    in_maps = _make_in_maps(**inputs)
    res = run_bass_kernel_spmd(
        nc, in_maps, core_ids=list(range(NCORES)), trace=trace
    )
    out_full = np.empty((B, N, DIM), dtype=np.float32)
    for i in range(NCORES):
        g, r = i // GROUP, i % GROUP
        out_full[g, r * NQ : (r + 1) * NQ, :] = (
            res.results[i]["out"].astype(np.float32).T
        )
    return out_full, res


def kernel(**inputs) -> np.ndarray:
    out, _ = run(inputs, trace=False)
    return out


# revision 26
# speedup vs baseline: 1.2763x; 1.2763x over previous
"""Distributed Trainium2 Bass kernel for pre-LN multi-head attention (v3).

Reference computation (per batch b of 2, seq n=2048, dim=1024, 16 heads x 64):
    xn = LayerNorm(x) * gamma + beta
    q, k = split(xn @ W_qk); v = xn @ W_v
    out = softmax(q k^T / 8) v  (per head)
    y = out @ W_out + b_out

Sharding (head-parallel attention): 8 cores = 2 batch groups x 4 head groups.
Core i owns batch g=i//4 and heads [4r, 4r+4) with r=i%4. Each core receives
the FULL batch x (host-cast bf16, token-rotated, see below), runs LayerNorm
over all 2048 tokens (4x redundant but cheap, hides under DMA), computes
K^T/V/Q projections for its 4 heads over all tokens (same PE cycles as a
token-sharded projection), and runs attention for its 4 heads over all 2048
queries x 2048 keys. No mid-kernel K/V AllGather: that collective stream only
starts after the ~55us CC mesh bring-up and runs at ~75GB/s, costing ~50us of
PE idle in the token-sharded design.

The ACT engine's exp stream is the binding resource of the attention phase
(16.8M exps at 1 elem/cycle/lane ~= 140us); everything is organized to start
it as early as possible and keep it dense:
- proj emission order K, V, Q(heads 0-1), attention units (h0, h1) of the
  first quarter, Q(heads 2-3), rest — so the first S^T/exp work reaches the
  PE/ACT queues ~40us before the projections fully drain.
- PSUM pools are shared between proj chains and attention (pp + ps_s halves)
  so attention pools coexist with proj pools without exceeding 8 banks.

Communication is a late attention-output exchange, after the CC mesh is up.
The host rotates each core's tokens by (r+1)*512 so program position-quarter
d = absolute quarter (r+1+d)%4 and position 3 is the core's OWN quarter:
foreign quarters finish first and AllGather (one 256KB-in op per quarter)
within the batch group mid-attention, fully hidden; the own quarter needs no
exchange. Each core extracts, from gather d, the tile of the one peer whose
position-d is this core's absolute quarter — a single DMA whose source offset
is computed at runtime from partition_id() (rank-dependent extraction in a
single SPMD program). Tiles land in att_full in relative-slot order; W_out
rows are host-rotated to match, so the output projection (full 16-head
contraction for the own 512-token quarter) is entirely static. y^T is written
directly; the host assembles 8 [1024, 512] shards, no reduction.
"""
import sys
import types

sys.path.insert(0, "/opt/trn_rl_repo")

# Register the NTFF profile hook that trn_boot skips when the image's antenv
# lacks axon_hooks, so run_bass_kernel_spmd(trace=True) can report exec time.
if "antenv.axon_hooks" not in sys.modules:
    try:
        from trn_agent_boot.trn_boot import _ntff_profile_via_ctypes

        _hook = _ntff_profile_via_ctypes("/opt/axon/libaxon_pjrt.so")
    except Exception:
        _hook = None
    _mod = types.ModuleType("antenv.axon_hooks")
    _mod.get_axon_ntff_profile_hook = lambda: _hook
    _mod.set_axon_ntff_profile_hook = lambda h: None
    sys.modules["antenv.axon_hooks"] = _mod

from contextlib import ExitStack

import ml_dtypes
import numpy as np
import concourse.bass as bass
import concourse.tile as tile
from concourse import bacc, mybir
from concourse.bass_utils import run_bass_kernel_spmd
from concourse.masks import make_identity

F32 = mybir.dt.float32
BF16 = mybir.dt.bfloat16
AF = mybir.ActivationFunctionType
ALU = mybir.AluOpType

B, N, DIM = 2, 2048, 1024
HEADS, DH = 16, 64
INNER = HEADS * DH  # 1024
SCALE = DH**-0.5
EPS = 1e-5

NCORES = 8
GROUP = 4          # cores per batch group (head groups / output quarters)
LH = HEADS // GROUP  # 4 local heads per core
LIN = LH * DH      # 256 local inner dims
NQ = N // GROUP    # 512 tokens per output quarter
DC = DIM // 128    # 8 dim chunks
KCH = N // 128     # 16 key chunks of 128 tokens
KPAIR = KCH // 2   # exp batches of 2 key chunks
TG = 4             # token groups of 512 for LN/proj pipeline

MMDT = BF16        # matmul operand storage dtype

REPLICA_GROUPS = [[0, 1, 2, 3], [4, 5, 6, 7]]

VSTR = 448         # per-key-chunk vones stride: 2x[ones64|V_2c|V_2c+1] + ones64
# DVE "exp": u = round(a*logit + (127*128 - c)); the int16 bit pattern of u
# IS bf16(exp(logit)) up to the linear-interp-between-octaves error (~1.5%
# rms, validated end-to-end at rel_err 1.14e-2 vs the 2e-2 gate when applied
# to half the key chunks). psum holds 2*S so a absorbs SCALE/2.
SCH_A = (128.0 / float(np.log(2.0))) * (SCALE / 2.0)
SCH_B = 127.0 * 128.0 - 8.0
QSZ = 2 * 128 * NQ  # one staged quarter tile (both head pairs), flat


def build_nc():
    nc = bacc.Bacc(num_devices=NCORES)

    x = nc.dram_tensor("x", [N, DIM], MMDT, kind="ExternalInput")
    gbb = nc.dram_tensor("gbb", [128, 24], F32, kind="ExternalInput")
    wqk_k = nc.dram_tensor("wqk_k", [DIM, LIN], MMDT, kind="ExternalInput")
    wqk_q = nc.dram_tensor("wqk_q", [DIM, LIN], MMDT, kind="ExternalInput")
    w_v = nc.dram_tensor("W_v", [DIM, LIN], MMDT, kind="ExternalInput")
    w_out = nc.dram_tensor("W_out", [INNER, DIM], MMDT, kind="ExternalInput")
    out = nc.dram_tensor("out", [DIM, NQ], MMDT, kind="ExternalOutput")

    with tile.TileContext(nc) as tc, ExitStack() as ctx:
        pool = lambda name, bufs, **kw: ctx.enter_context(
            tc.tile_pool(name=name, bufs=bufs, **kw)
        )
        consts = pool("consts", 1)
        dram = pool("dram", 1, space="DRAM")
        kv = pool("kv", 1)          # kt_sb + vones + q_t (live whole kernel)
        att_pool = pool("att", 1)
        wo_pool = pool("wo", 1)
        small = pool("small", 8)
        es_pool = pool("es", 8)
        es16_pool = pool("es16", 6)
        rp_pool = pool("rp", 2)
        y_pool = pool("y", 2)
        pp = pool("pp", 2, space="PSUM")      # acc chains / AV accumulators

        # ---- constants ---------------------------------------------------
        gbb_t = consts.tile([128, 24], F32)   # [gamma | beta | b_out] per c
        nc.sync.dma_start(gbb_t[:], gbb[:, :])
        ident = consts.tile([128, 128], MMDT)
        make_identity(nc, ident[:])
        eps_sb = consts.tile([128, 1], F32)
        nc.vector.memset(eps_sb[:], EPS)
        # PE warmup: ramp the clock while input DMAs are in flight.
        wps = pp.tile([128, 512], F32, tag="acc", name="warmup")
        for i in range(24):
            nc.tensor.matmul(
                wps[:, 0:128], ident[:], ident[:], start=(i == 0), stop=(i == 23)
            )

        cc_ins = [dram.tile([QSZ], MMDT, name=f"cc_in{d}") for d in range(3)]
        cc_outs = [
            dram.tile([GROUP * QSZ], MMDT, name=f"cc_out{d}") for d in range(3)
        ]

        # K^T per head, duplicated across both 64-row halves so the S^T
        # matmuls contract over K=128 (computing 2*S, folded into the exp
        # scale; K=64 matmuls were observed to hold the HAM clock at 1.2GHz).
        kt_sb = kv.tile([128, LH * N], MMDT)
        # V interleaved with ones blocks: key chunk kc spans [kc*448, +448):
        # [ones64 | V_h0 | V_h1 | ones64 | V_h2 | V_h3 | ones64]. Head h's AV
        # lhsT = cols kc*448 + (h//2)*192 + (h%2)*128, len 128: even heads
        # [ones | V] (AV rows 0:64 = colsum, 64:128 = data), odd swapped.
        vones = kv.tile([128, KCH * VSTR], MMDT)
        # Q^T duplicated per head like K^T (see kt_sb note).
        q_t = kv.tile([128, LH * N], MMDT)
        # attention output^T: position-quarter d, head pair hc at chunk
        # (d*2 + hc), [128, 512] with head parity on the 64-row halves.
        att_t = att_pool.tile([128, GROUP * 2 * NQ], MMDT)
        wout_sb = wo_pool.tile([128, DC * DIM], MMDT)
        att_full = wo_pool.tile([128, DC * NQ], MMDT)

        for kc in range(KCH):
            ones_base = vones[:, kc * VSTR : kc * VSTR + 64]
            nc.vector.memset(
                bass.AP(
                    tensor=ones_base.tensor,
                    offset=ones_base.offset,
                    ap=[ones_base.ap[0], [192, 3], [1, 64]],
                ),
                1.0,
            )

        # ---- attention helpers (outer pools only) ------------------------
        def av_lhs(h, kc):
            base = kc * VSTR + (h // 2) * 192 + (h % 2) * 128
            return vones[:, base : base + 128]

        def head_divide(h, d, po):
            hp = (h % 2) * 64
            hc = h // 2
            cb, dp = hp, 64 - hp
            recip = rp_pool.tile([128, 1024], F32, tag="recip", name="recip")
            nc.vector.tensor_copy(recip[0:64, 512:1024], po[cb : cb + 64, :])
            nc.vector.reciprocal_approx_fast(
                recip[0:64, 0:512], recip[0:64, 512:1024]
            )
            nc.vector.tensor_mul(
                att_t[hp : hp + 64, (d * 2 + hc) * NQ : (d * 2 + hc + 1) * NQ],
                po[dp : dp + 64, :],
                recip[0:64, 0:512],
            )

        def head_unit(ps_pool, h, d):
            # interleaved S/exp/AV pipeline for one (head, position-quarter);
            # exp alternates between ACT (exact) and DVE (Schraudolph bf16)
            # so neither engine is the wall
            po = pp.tile([128, 512], F32, tag="acc", name="po")
            for pr in range(KPAIR):
                pss = ps_pool.tile([128, 1024], F32, tag="pss", name="pss")
                for j in range(2):
                    kc = 2 * pr + j
                    nc.tensor.matmul(
                        pss[:, j * 512 : (j + 1) * 512],
                        kt_sb[:, h * N + kc * 128 : h * N + (kc + 1) * 128],
                        q_t[:, h * N + d * 512 : h * N + (d + 1) * 512],
                        start=True,
                        stop=True,
                    )
                if pr % 2 == 0:
                    es = es_pool.tile([128, 1024], MMDT, tag="es", name="es")
                    # psum holds 2*S (duplicated operands) -> halve the scale
                    nc.scalar.activation(es[:], pss[:], AF.Exp, scale=SCALE / 2)
                    esv = es[:]
                else:
                    e16 = es16_pool.tile(
                        [128, 1024], mybir.dt.int16, tag="e16", name="e16"
                    )
                    nc.vector.tensor_scalar(
                        out=e16[:],
                        in0=pss[:],
                        scalar1=SCH_A,
                        scalar2=SCH_B,
                        op0=ALU.mult,
                        op1=ALU.add,
                    )
                    esv = e16[:].bitcast(MMDT)
                for j in range(2):
                    kc = 2 * pr + j
                    nc.tensor.matmul(
                        po[:],
                        av_lhs(h, kc),
                        esv[:, j * 512 : (j + 1) * 512],
                        start=(pr == 0 and j == 0),
                        stop=(pr == KPAIR - 1 and j == 1),
                    )
            head_divide(h, d, po)

        sp = nc.engines[mybir.EngineType.SP]

        def stage_and_gather(d):
            # stage position-quarter d's full tile (both head pairs), then
            # AllGather it within the batch group
            nc.sync.dma_start(
                cc_ins[d][:].rearrange("(p f) -> p f", f=2 * NQ),
                att_t[:, (d * 2) * NQ : (d * 2 + 2) * NQ],
            )
            nc.gpsimd.collective_compute(
                "AllGather",
                ALU.bypass,
                replica_groups=REPLICA_GROUPS,
                ins=[cc_ins[d][:].opt()],
                outs=[cc_outs[d][:].opt()],
            )

        def extract(d, rid):
            # gather d carries every rank's position-d tile; MY quarter's
            # tile in it comes from rank (rid-1-d)%4 (the peer whose
            # position-d is my absolute quarter). It lands in att_full
            # relative slot d+1 (static) -- W_out rows are host-rotated to
            # the relative-slot order.
            roff = sp.compute_val(((rid + 3 - d) % GROUP) * QSZ)
            blk = cc_outs[d][bass.ds(roff, QSZ)]
            src = bass.AP(
                tensor=blk.tensor,
                offset=blk.offset,
                ap=[[2 * NQ, 128], [1, 2 * NQ]],
            )
            nc.sync.dma_start(
                att_full[:, (d + 1) * 2 * NQ : (d + 2) * 2 * NQ], src
            )

        def emit_wout(cs):
            for c in cs:
                nc.sync.dma_start(
                    wout_sb[:, c * DIM : (c + 1) * DIM],
                    w_out[c * 128 : (c + 1) * 128, :],
                )

        # ---- LN + projections --------------------------------------------
        with ExitStack() as proj_ctx:
            ppool = lambda name, bufs, **kw: proj_ctx.enter_context(
                tc.tile_pool(name=name, bufs=bufs, **kw)
            )
            ptr = ppool("ptr", 2, space="PSUM")  # transpose targets
            pkv = ppool("pkv", 2, space="PSUM")  # second accumulation chain
            xw = ppool("xw", 1)
            x_pool = ppool("xp", 8)
            xn_pool = ppool("xn", 2)
            vt_pool = ppool("vt", 2)
            xnt = xw.tile([128, DC * N], MMDT)
            wk_sb = xw.tile([128, DC * LIN], MMDT)
            wv_sb = xw.tile([128, DC * LIN], MMDT)
            wq_sb = xw.tile([128, DC * LIN], MMDT)

            # x chunk tiles from a rotating pool; DMAs emitted in
            # consumption order interleaved with the weight loads,
            # <=256KB per DMA.
            x_tiles = []

            def emit_x(ts):
                for t in ts:
                    xt = x_pool.tile([128, DIM], MMDT, tag="x", name=f"x{t}")
                    nc.sync.dma_start(xt[:], x[t * 128 : (t + 1) * 128, :])
                    x_tiles.append(xt)

            emit_x(range(0, 8))
            for w_sb, w_hbm in ((wk_sb, wqk_k), (wv_sb, w_v), (wq_sb, wqk_q)):
                for c in range(DC):
                    nc.sync.dma_start(
                        w_sb[:, c * LIN : (c + 1) * LIN],
                        w_hbm[c * 128 : (c + 1) * 128, :],
                    )
            emit_x(range(8, 16))

            def ln_group(tg):
                xn_t = xn_pool.tile([128, 4 * DIM], MMDT, tag="xn")
                for i in range(4):
                    xt = x_tiles[tg * 4 + i][:]
                    xg = xt.rearrange("p (n s) -> p n s", s=512)
                    stats = small.tile([128, 2, 6], F32)
                    for sgi in range(2):
                        nc.vector.bn_stats(stats[:, sgi, :], xg[:, sgi, :])
                    mv = small.tile([128, 2], F32)
                    nc.vector.bn_aggr(mv[:], stats[:])
                    rstd = small.tile([128, 1], F32)
                    nc.scalar.activation(rstd[:], mv[:, 1:2], AF.Sqrt, bias=eps_sb[:])
                    nc.vector.reciprocal(rstd[:], rstd[:])
                    nc.vector.tensor_scalar(
                        out=xn_t[:, i * DIM : (i + 1) * DIM],
                        in0=xt,
                        scalar1=mv[:, 0:1],
                        scalar2=rstd[:],
                        op0=ALU.subtract,
                        op1=ALU.mult,
                    )
                # transpose to [dim, tokens], fusing gamma/beta; split the
                # scale/cast between Scalar and Vector engines
                for c in range(DC):
                    pt = ptr.tile([128, 512], MMDT, tag="tr")
                    for i in range(4):
                        nc.tensor.transpose(
                            pt[:, i * 128 : (i + 1) * 128],
                            xn_t[:, i * DIM + c * 128 : i * DIM + (c + 1) * 128],
                            ident[:],
                        )
                    dst = xnt[:, c * N + tg * 512 : c * N + (tg + 1) * 512]
                    if (c + tg) % 2 == 0:
                        nc.scalar.activation(
                            dst,
                            pt[:],
                            AF.Identity,
                            bias=gbb_t[:, 8 + c : 9 + c],
                            scale=gbb_t[:, c : c + 1],
                        )
                    else:
                        nc.vector.tensor_scalar(
                            out=dst,
                            in0=pt[:],
                            scalar1=gbb_t[:, c : c + 1],
                            scalar2=gbb_t[:, 8 + c : 9 + c],
                            op0=ALU.mult,
                            op1=ALU.add,
                        )

            def two_chains(width=512):
                c0 = pp.tile([128, 512], F32, tag="acc", name="c0")
                c1 = pkv.tile([128, 512], F32, tag="kvacc", name="c1")
                return [c0[:, 0:width], c1[:, 0:width]]

            def k_proj(s):
                pqs = two_chains()
                for c in range(DC):
                    for m in range(2):
                        nc.tensor.matmul(
                            pqs[m],
                            wk_sb[:, c * LIN + m * 128 : c * LIN + (m + 1) * 128],
                            xnt[:, c * N + s * 512 : c * N + (s + 1) * 512],
                            start=(c == 0),
                            stop=(c == DC - 1),
                        )
                for m in range(2):
                    for lh in range(2):
                        h = 2 * m + lh
                        span = slice(h * N + s * 512, h * N + (s + 1) * 512)
                        src = pqs[m][lh * 64 : lh * 64 + 64, :]
                        if (m + lh) % 2 == 0:
                            nc.vector.tensor_copy(kt_sb[0:64, span], src)
                        else:
                            nc.scalar.copy(kt_sb[0:64, span], src)
                        # K=128 duplication: SBUF->SBUF DMA, no HBM traffic
                        nc.sync.dma_start(kt_sb[64:128, span], kt_sb[0:64, span])

            def v_proj(s):
                for tp in range(2):
                    pvs = two_chains(width=LIN)
                    for c in range(DC):
                        for ti in range(2):
                            t = s * 4 + 2 * tp + ti
                            nc.tensor.matmul(
                                pvs[ti],
                                xnt[:, c * N + t * 128 : c * N + (t + 1) * 128],
                                wv_sb[:, c * LIN : (c + 1) * LIN],
                                start=(c == 0),
                                stop=(c == DC - 1),
                            )
                    for ti in range(2):
                        kc = s * 4 + 2 * tp + ti
                        vdst = vones[:, kc * VSTR + 64 :]
                        dst_ap = bass.AP(
                            tensor=vdst.tensor,
                            offset=vdst.offset,
                            ap=[vdst.ap[0], [192, 2], [1, 128]],
                        )
                        srcv = pvs[ti].rearrange("p (n f) -> p n f", f=128)
                        if ti == 0:
                            nc.vector.tensor_copy(dst_ap, srcv)
                        else:
                            nc.scalar.copy(dst_ap, srcv)

            def q_proj_m(m):
                # head pair m over all 4 spans: 4 interleaved chains
                c0 = pp.tile([128, 512], F32, tag="acc", name="q0")
                c1 = pp.tile([128, 512], F32, tag="acc", name="q1")
                c2 = pkv.tile([128, 512], F32, tag="kvacc", name="q2")
                c3 = pkv.tile([128, 512], F32, tag="kvacc", name="q3")
                pqs = [c0[:], c1[:], c2[:], c3[:]]
                for c in range(DC):
                    for s in range(TG):
                        nc.tensor.matmul(
                            pqs[s],
                            wq_sb[:, c * LIN + m * 128 : c * LIN + (m + 1) * 128],
                            xnt[:, c * N + s * 512 : c * N + (s + 1) * 512],
                            start=(c == 0),
                            stop=(c == DC - 1),
                        )
                for s in range(TG):
                    for lh in range(2):
                        h = 2 * m + lh
                        span = slice(h * N + s * 512, h * N + (s + 1) * 512)
                        src = pqs[s][lh * 64 : lh * 64 + 64, :]
                        if (s + lh) % 2 == 0:
                            nc.vector.tensor_copy(q_t[0:64, span], src)
                        else:
                            nc.scalar.copy(q_t[0:64, span], src)
                        nc.sync.dma_start(q_t[64:128, span], q_t[0:64, span])

            for tg in range(TG):
                ln_group(tg)
            for s in range(TG):
                k_proj(s)
            for s in range(TG):
                v_proj(s)
            def early_unit(h, d):
                # first-quarter unit emitted inside the proj scope, S^T in
                # [128, 512] singles from the pkv pool: the ACT exp stream
                # (the binding resource) starts ~40us before the remaining
                # projections drain
                po = pp.tile([128, 512], F32, tag="acc", name="po")
                for kc in range(KCH):
                    pss = pkv.tile([128, 512], F32, tag="kvacc", name="pse")
                    nc.tensor.matmul(
                        pss[:],
                        kt_sb[:, h * N + kc * 128 : h * N + (kc + 1) * 128],
                        q_t[:, h * N + d * 512 : h * N + (d + 1) * 512],
                        start=True,
                        stop=True,
                    )
                    if kc % 2 == 0:
                        es = es_pool.tile([128, 1024], MMDT, tag="es", name="es")
                        nc.scalar.activation(
                            es[:, 0:512], pss[:], AF.Exp, scale=SCALE / 2
                        )
                        esv = es[:, 0:512]
                    else:
                        e16 = es16_pool.tile(
                            [128, 1024], mybir.dt.int16, tag="e16", name="e16"
                        )
                        nc.vector.tensor_scalar(
                            out=e16[:, 0:512],
                            in0=pss[:],
                            scalar1=SCH_A,
                            scalar2=SCH_B,
                            op0=ALU.mult,
                            op1=ALU.add,
                        )
                        esv = e16[:, 0:512].bitcast(MMDT)
                    nc.tensor.matmul(
                        po[:],
                        av_lhs(h, kc),
                        esv,
                        start=(kc == 0),
                        stop=(kc == KCH - 1),
                    )
                head_divide(h, d, po)

            # Q heads 0-1, then the first four attention units (heads 0-1
            # of positions 0 and 1 need only Q chunk m0); their exp backlog
            # covers the Qm1 projection with no ACT gap
            q_proj_m(0)
            early_unit(0, 0)
            early_unit(1, 0)
            q_proj_m(1)

        # ---- attention + exchange ----------------------------------------
        ps_s = ctx.enter_context(
            tc.tile_pool(name="ps_s", bufs=3, space="PSUM")
        )
        rid = sp.partition_id() % GROUP

        # quarter-major; position d = absolute quarter (rid+1+d)%4 via the
        # host token rotation, so position 3 is my OWN quarter and needs no
        # exchange. Foreign quarters gather mid-attention.
        for d in range(GROUP):
            for h in range(LH):
                if d == 0 and h < 2:
                    continue  # emitted inside proj_ctx above
                head_unit(ps_s, h, d)
            if d < 3:
                stage_and_gather(d)
                # spread the W_out loads (2MB total) across the quarter
                # boundaries; a single 8-chunk burst was observed to stretch
                # concurrent ACT exp instructions
                emit_wout(range(2 + d * 2, 4 + d * 2))
        emit_wout([0, 1])
        # own tile -> relative slot 0
        nc.sync.dma_start(att_full[:, 0 : 2 * NQ], att_t[:, 6 * NQ : 8 * NQ])
        for d in range(3):
            extract(d, rid)

        # ---- output projection y^T = W_out^T @ att^T + b_out -------------
        # four interleaved accumulation chains; foreign att chunks (landed
        # mid-attention) accumulate first, own chunks last
        C_ORDER = [2, 3, 4, 5, 6, 7, 0, 1]
        for mp in range(DC // 4):
            pys = []
            for mi in range(4):
                if mi < 2:
                    py = pp.tile([128, 512], F32, tag="acc", name=f"py{mi}")
                    pys.append(py[:])
                else:
                    py = ps_s.tile([128, 1024], F32, tag="pss", name=f"py{mi}")
                    pys.append(py[:, 0:512])
            for ci, c in enumerate(C_ORDER):
                for mi in range(4):
                    m = 4 * mp + mi
                    nc.tensor.matmul(
                        pys[mi],
                        wout_sb[:, c * DIM + m * 128 : c * DIM + (m + 1) * 128],
                        att_full[:, c * NQ : (c + 1) * NQ],
                        start=(ci == 0),
                        stop=(ci == DC - 1),
                    )
            for mi in range(4):
                m = 4 * mp + mi
                y_sb = y_pool.tile([128, 512], MMDT, tag="y")
                nc.vector.tensor_scalar(
                    out=y_sb[:],
                    in0=pys[mi],
                    scalar1=gbb_t[:, 16 + m : 17 + m],
                    scalar2=None,
                    op0=ALU.add,
                )
                nc.sync.dma_start(out[m * 128 : (m + 1) * 128, :], y_sb[:])

    nc.compile()
    return nc


_NC_CACHE = None


def _get_nc():
    global _NC_CACHE
    if _NC_CACHE is None:
        _NC_CACHE = build_nc()
    return _NC_CACHE


def _make_in_maps(x, ln_gamma, ln_beta, W_qk, W_v, W_out, b_out):
    mmnp = mybir.dt.np(MMDT)
    wqk = np.asarray(W_qk, dtype=np.float32)
    wv = np.asarray(W_v, dtype=np.float32)
    wo = np.asarray(W_out, dtype=np.float32)
    gamma = np.asarray(ln_gamma, dtype=np.float32).reshape(DC, 128).T
    beta = np.asarray(ln_beta, dtype=np.float32).reshape(DC, 128).T
    bout = np.asarray(b_out, dtype=np.float32).reshape(DC, 128).T
    gbb = np.ascontiguousarray(np.concatenate([gamma, beta, bout], axis=1))
    xf = np.asarray(x, dtype=np.float32)
    xb = [np.ascontiguousarray(xf[g]).astype(mmnp) for g in range(B)]
    in_maps = []
    for i in range(NCORES):
        g, r = i // GROUP, i % GROUP
        cols = slice(r * LIN, (r + 1) * LIN)
        kcols = slice(INNER + r * LIN, INNER + (r + 1) * LIN)
        # token rotation: program position-quarter d = absolute quarter
        # (r+1+d)%4, so position 3 is this core's own quarter
        xr = np.roll(xb[g], -(r + 1) * NQ, axis=0)
        # W_out rows in relative-slot order: slot s holds the heads of
        # rank (r-s)%4 (slot 0 = own heads)
        wor = np.concatenate(
            [
                wo[((r - s) % GROUP) * LIN : (((r - s) % GROUP) + 1) * LIN]
                for s in range(GROUP)
            ]
        )
        in_maps.append(
            {
                "x": np.ascontiguousarray(xr),
                "gbb": gbb,
                "wqk_k": np.ascontiguousarray(wqk[:, kcols]).astype(mmnp),
                "wqk_q": np.ascontiguousarray(wqk[:, cols]).astype(mmnp),
                "W_v": np.ascontiguousarray(wv[:, cols]).astype(mmnp),
                "W_out": np.ascontiguousarray(wor).astype(mmnp),
            }
        )
    return in_maps


def run(inputs: dict, trace: bool = False):
    """Run the distributed kernel; returns (full_output, BassKernelResults)."""
    nc = _get_nc()
    in_maps = _make_in_maps(**inputs)
    res = run_bass_kernel_spmd(
        nc, in_maps, core_ids=list(range(NCORES)), trace=trace
    )
    out_full = np.empty((B, N, DIM), dtype=np.float32)
    for i in range(NCORES):
        g, r = i // GROUP, i % GROUP
        out_full[g, r * NQ : (r + 1) * NQ, :] = (
            res.results[i]["out"].astype(np.float32).T
        )
    return out_full, res


def kernel(**inputs) -> np.ndarray:
    out, _ = run(inputs, trace=False)
    return out


# revision 27
# speedup vs baseline: 1.2900x; 1.0107x over previous
"""Distributed Trainium2 Bass kernel for pre-LN multi-head attention (v3).

Reference computation (per batch b of 2, seq n=2048, dim=1024, 16 heads x 64):
    xn = LayerNorm(x) * gamma + beta
    q, k = split(xn @ W_qk); v = xn @ W_v
    out = softmax(q k^T / 8) v  (per head)
    y = out @ W_out + b_out

Sharding (head-parallel attention): 8 cores = 2 batch groups x 4 head groups.
Core i owns batch g=i//4 and heads [4r, 4r+4) with r=i%4. Each core receives
the FULL batch x (host-cast bf16, token-rotated, see below), runs LayerNorm
over all 2048 tokens (4x redundant but cheap, hides under DMA), computes
K^T/V/Q projections for its 4 heads over all tokens (same PE cycles as a
token-sharded projection), and runs attention for its 4 heads over all 2048
queries x 2048 keys. No mid-kernel K/V AllGather: that collective stream only
starts after the ~55us CC mesh bring-up and runs at ~75GB/s, costing ~50us of
PE idle in the token-sharded design.

The ACT engine's exp stream is the binding resource of the attention phase
(16.8M exps at 1 elem/cycle/lane ~= 140us); everything is organized to start
it as early as possible and keep it dense:
- proj emission order K, V, Q(heads 0-1), attention units (h0, h1) of the
  first quarter, Q(heads 2-3), rest — so the first S^T/exp work reaches the
  PE/ACT queues ~40us before the projections fully drain.
- PSUM pools are shared between proj chains and attention (pp + ps_s halves)
  so attention pools coexist with proj pools without exceeding 8 banks.

Communication is a late attention-output exchange, after the CC mesh is up.
The host rotates each core's tokens by (r+1)*512 so program position-quarter
d = absolute quarter (r+1+d)%4 and position 3 is the core's OWN quarter:
foreign quarters finish first and AllGather (one 256KB-in op per quarter)
within the batch group mid-attention, fully hidden; the own quarter needs no
exchange. Each core extracts, from gather d, the tile of the one peer whose
position-d is this core's absolute quarter — a single DMA whose source offset
is computed at runtime from partition_id() (rank-dependent extraction in a
single SPMD program). Tiles land in att_full in relative-slot order; W_out
rows are host-rotated to match, so the output projection (full 16-head
contraction for the own 512-token quarter) is entirely static. y^T is written
directly; the host assembles 8 [1024, 512] shards, no reduction.
"""
import sys
import types

sys.path.insert(0, "/opt/trn_rl_repo")

# Register the NTFF profile hook that trn_boot skips when the image's antenv
# lacks axon_hooks, so run_bass_kernel_spmd(trace=True) can report exec time.
if "antenv.axon_hooks" not in sys.modules:
    try:
        from trn_agent_boot.trn_boot import _ntff_profile_via_ctypes

        _hook = _ntff_profile_via_ctypes("/opt/axon/libaxon_pjrt.so")
    except Exception:
        _hook = None
    _mod = types.ModuleType("antenv.axon_hooks")
    _mod.get_axon_ntff_profile_hook = lambda: _hook
    _mod.set_axon_ntff_profile_hook = lambda h: None
    sys.modules["antenv.axon_hooks"] = _mod

from contextlib import ExitStack

import ml_dtypes
import numpy as np
import concourse.bass as bass
import concourse.tile as tile
from concourse import bacc, mybir
from concourse.bass_utils import run_bass_kernel_spmd
from concourse.masks import make_identity

F32 = mybir.dt.float32
BF16 = mybir.dt.bfloat16
AF = mybir.ActivationFunctionType
ALU = mybir.AluOpType

B, N, DIM = 2, 2048, 1024
HEADS, DH = 16, 64
INNER = HEADS * DH  # 1024
SCALE = DH**-0.5
EPS = 1e-5

NCORES = 8
GROUP = 4          # cores per batch group (head groups / output quarters)
LH = HEADS // GROUP  # 4 local heads per core
LIN = LH * DH      # 256 local inner dims
NQ = N // GROUP    # 512 tokens per output quarter
DC = DIM // 128    # 8 dim chunks
KCH = N // 128     # 16 key chunks of 128 tokens
KPAIR = KCH // 2   # exp batches of 2 key chunks
TG = 4             # token groups of 512 for LN/proj pipeline

MMDT = BF16        # matmul operand storage dtype

REPLICA_GROUPS = [[0, 1, 2, 3], [4, 5, 6, 7]]

VSTR = 448         # per-key-chunk vones stride: 2x[ones64|V_2c|V_2c+1] + ones64
# DVE "exp": u = round(a*logit + (127*128 - c)); the int16 bit pattern of u
# IS bf16(exp(logit)) up to the linear-interp-between-octaves error (~1.5%
# rms, validated end-to-end at rel_err 1.14e-2 vs the 2e-2 gate when applied
# to half the key chunks). psum holds 2*S so a absorbs SCALE/2.
SCH_A = (128.0 / float(np.log(2.0))) * (SCALE / 2.0)
SCH_B = 127.0 * 128.0 - 8.0
QSZ = 2 * 128 * NQ  # one staged quarter tile (both head pairs), flat


def build_nc():
    nc = bacc.Bacc(num_devices=NCORES)

    x = nc.dram_tensor("x", [N, DIM], MMDT, kind="ExternalInput")
    gbb = nc.dram_tensor("gbb", [128, 24], F32, kind="ExternalInput")
    wqk_k = nc.dram_tensor("wqk_k", [DIM, LIN], MMDT, kind="ExternalInput")
    wqk_q = nc.dram_tensor("wqk_q", [DIM, LIN], MMDT, kind="ExternalInput")
    w_v = nc.dram_tensor("W_v", [DIM, LIN], MMDT, kind="ExternalInput")
    w_out = nc.dram_tensor("W_out", [INNER, DIM], MMDT, kind="ExternalInput")
    out = nc.dram_tensor("out", [DIM, NQ], MMDT, kind="ExternalOutput")

    with tile.TileContext(nc) as tc, ExitStack() as ctx:
        pool = lambda name, bufs, **kw: ctx.enter_context(
            tc.tile_pool(name=name, bufs=bufs, **kw)
        )
        consts = pool("consts", 1)
        dram = pool("dram", 1, space="DRAM")
        kv = pool("kv", 1)          # kt_sb + vones + q_t (live whole kernel)
        att_pool = pool("att", 1)
        wo_pool = pool("wo", 1)
        small = pool("small", 8)
        es_pool = pool("es", 8)
        es16_pool = pool("es16", 6)
        rp_pool = pool("rp", 2)
        y_pool = pool("y", 2)
        pp = pool("pp", 2, space="PSUM")      # acc chains / AV accumulators

        # ---- constants ---------------------------------------------------
        gbb_t = consts.tile([128, 24], F32)   # [gamma | beta | b_out] per c
        nc.sync.dma_start(gbb_t[:], gbb[:, :])
        ident = consts.tile([128, 128], MMDT)
        make_identity(nc, ident[:])
        eps_sb = consts.tile([128, 1], F32)
        nc.vector.memset(eps_sb[:], EPS)
        # PE warmup: ramp the clock while input DMAs are in flight.
        wps = pp.tile([128, 512], F32, tag="acc", name="warmup")
        for i in range(24):
            nc.tensor.matmul(
                wps[:, 0:128], ident[:], ident[:], start=(i == 0), stop=(i == 23)
            )

        cc_ins = [dram.tile([QSZ], MMDT, name=f"cc_in{d}") for d in range(3)]
        cc_outs = [
            dram.tile([GROUP * QSZ], MMDT, name=f"cc_out{d}") for d in range(3)
        ]

        # K^T per head, duplicated across both 64-row halves so the S^T
        # matmuls contract over K=128 (computing 2*S, folded into the exp
        # scale; K=64 matmuls were observed to hold the HAM clock at 1.2GHz).
        kt_sb = kv.tile([128, LH * N], MMDT)
        # V interleaved with ones blocks: key chunk kc spans [kc*448, +448):
        # [ones64 | V_h0 | V_h1 | ones64 | V_h2 | V_h3 | ones64]. Head h's AV
        # lhsT = cols kc*448 + (h//2)*192 + (h%2)*128, len 128: even heads
        # [ones | V] (AV rows 0:64 = colsum, 64:128 = data), odd swapped.
        vones = kv.tile([128, KCH * VSTR], MMDT)
        # Q^T duplicated per head like K^T (see kt_sb note).
        q_t = kv.tile([128, LH * N], MMDT)
        # attention output^T: position-quarter d, head pair hc at chunk
        # (d*2 + hc), [128, 512] with head parity on the 64-row halves.
        att_t = att_pool.tile([128, GROUP * 2 * NQ], MMDT)
        wout_sb = wo_pool.tile([128, DC * DIM], MMDT)
        att_full = wo_pool.tile([128, DC * NQ], MMDT)

        for kc in range(KCH):
            ones_base = vones[:, kc * VSTR : kc * VSTR + 64]
            nc.vector.memset(
                bass.AP(
                    tensor=ones_base.tensor,
                    offset=ones_base.offset,
                    ap=[ones_base.ap[0], [192, 3], [1, 64]],
                ),
                1.0,
            )

        # ---- attention helpers (outer pools only) ------------------------
        def av_lhs(h, kc):
            base = kc * VSTR + (h // 2) * 192 + (h % 2) * 128
            return vones[:, base : base + 128]

        def head_divide(h, d, po):
            hp = (h % 2) * 64
            hc = h // 2
            cb, dp = hp, 64 - hp
            recip = rp_pool.tile([128, 1024], F32, tag="recip", name="recip")
            nc.vector.tensor_copy(recip[0:64, 512:1024], po[cb : cb + 64, :])
            nc.vector.reciprocal_approx_fast(
                recip[0:64, 0:512], recip[0:64, 512:1024]
            )
            nc.vector.tensor_mul(
                att_t[hp : hp + 64, (d * 2 + hc) * NQ : (d * 2 + hc + 1) * NQ],
                po[dp : dp + 64, :],
                recip[0:64, 0:512],
            )

        def head_unit(ps_pool, h, d):
            # interleaved S/exp/AV pipeline for one (head, position-quarter);
            # exp alternates between ACT (exact) and DVE (Schraudolph bf16)
            # so neither engine is the wall
            po = pp.tile([128, 512], F32, tag="acc", name="po")
            for pr in range(KPAIR):
                pss = ps_pool.tile([128, 1024], F32, tag="pss", name="pss")
                for j in range(2):
                    kc = 2 * pr + j
                    nc.tensor.matmul(
                        pss[:, j * 512 : (j + 1) * 512],
                        kt_sb[:, h * N + kc * 128 : h * N + (kc + 1) * 128],
                        q_t[:, h * N + d * 512 : h * N + (d + 1) * 512],
                        start=True,
                        stop=True,
                    )
                if pr % 2 == 0:
                    es = es_pool.tile([128, 1024], MMDT, tag="es", name="es")
                    # psum holds 2*S (duplicated operands) -> halve the scale
                    nc.scalar.activation(es[:], pss[:], AF.Exp, scale=SCALE / 2)
                    esv = es[:]
                else:
                    e16 = es16_pool.tile(
                        [128, 1024], mybir.dt.int16, tag="e16", name="e16"
                    )
                    nc.vector.tensor_scalar(
                        out=e16[:],
                        in0=pss[:],
                        scalar1=SCH_A,
                        scalar2=SCH_B,
                        op0=ALU.mult,
                        op1=ALU.add,
                    )
                    esv = e16[:].bitcast(MMDT)
                for j in range(2):
                    kc = 2 * pr + j
                    nc.tensor.matmul(
                        po[:],
                        av_lhs(h, kc),
                        esv[:, j * 512 : (j + 1) * 512],
                        start=(pr == 0 and j == 0),
                        stop=(pr == KPAIR - 1 and j == 1),
                    )
            head_divide(h, d, po)

        sp = nc.engines[mybir.EngineType.SP]

        def stage_and_gather(d):
            # stage position-quarter d's full tile (both head pairs), then
            # AllGather it within the batch group
            nc.sync.dma_start(
                cc_ins[d][:].rearrange("(p f) -> p f", f=2 * NQ),
                att_t[:, (d * 2) * NQ : (d * 2 + 2) * NQ],
            )
            nc.gpsimd.collective_compute(
                "AllGather",
                ALU.bypass,
                replica_groups=REPLICA_GROUPS,
                ins=[cc_ins[d][:].opt()],
                outs=[cc_outs[d][:].opt()],
            )

        def extract(d, rid):
            # gather d carries every rank's position-d tile; MY quarter's
            # tile in it comes from rank (rid-1-d)%4 (the peer whose
            # position-d is my absolute quarter). It lands in att_full
            # relative slot d+1 (static) -- W_out rows are host-rotated to
            # the relative-slot order.
            roff = sp.compute_val(((rid + 3 - d) % GROUP) * QSZ)
            blk = cc_outs[d][bass.ds(roff, QSZ)]
            src = bass.AP(
                tensor=blk.tensor,
                offset=blk.offset,
                ap=[[2 * NQ, 128], [1, 2 * NQ]],
            )
            nc.sync.dma_start(
                att_full[:, (d + 1) * 2 * NQ : (d + 2) * 2 * NQ], src
            )

        def emit_wout(cs):
            for c in cs:
                nc.sync.dma_start(
                    wout_sb[:, c * DIM : (c + 1) * DIM],
                    w_out[c * 128 : (c + 1) * 128, :],
                )

        # ---- LN + projections --------------------------------------------
        with ExitStack() as proj_ctx:
            ppool = lambda name, bufs, **kw: proj_ctx.enter_context(
                tc.tile_pool(name=name, bufs=bufs, **kw)
            )
            ptr = ppool("ptr", 2, space="PSUM")  # transpose targets
            pkv = ppool("pkv", 2, space="PSUM")  # second accumulation chain
            xw = ppool("xw", 1)
            x_pool = ppool("xp", 8)
            xn_pool = ppool("xn", 2)
            vt_pool = ppool("vt", 2)
            xnt = xw.tile([128, DC * N], MMDT)
            wk_sb = xw.tile([128, DC * LIN], MMDT)
            wv_sb = xw.tile([128, DC * LIN], MMDT)
            wq_sb = xw.tile([128, DC * LIN], MMDT)

            # x chunk tiles from a rotating pool; DMAs emitted in
            # consumption order interleaved with the weight loads,
            # <=256KB per DMA.
            x_tiles = []

            def emit_x(ts):
                for t in ts:
                    xt = x_pool.tile([128, DIM], MMDT, tag="x", name=f"x{t}")
                    nc.sync.dma_start(xt[:], x[t * 128 : (t + 1) * 128, :])
                    x_tiles.append(xt)

            emit_x(range(0, 8))
            for w_sb, w_hbm in ((wk_sb, wqk_k), (wv_sb, w_v), (wq_sb, wqk_q)):
                for c in range(DC):
                    nc.sync.dma_start(
                        w_sb[:, c * LIN : (c + 1) * LIN],
                        w_hbm[c * 128 : (c + 1) * 128, :],
                    )
            emit_x(range(8, 16))

            def ln_group(tg):
                xn_t = xn_pool.tile([128, 4 * DIM], MMDT, tag="xn")
                for i in range(4):
                    xt = x_tiles[tg * 4 + i][:]
                    xg = xt.rearrange("p (n s) -> p n s", s=512)
                    stats = small.tile([128, 2, 6], F32)
                    for sgi in range(2):
                        nc.vector.bn_stats(stats[:, sgi, :], xg[:, sgi, :])
                    mv = small.tile([128, 2], F32)
                    nc.vector.bn_aggr(mv[:], stats[:])
                    rstd = small.tile([128, 1], F32)
                    nc.scalar.activation(rstd[:], mv[:, 1:2], AF.Sqrt, bias=eps_sb[:])
                    nc.vector.reciprocal(rstd[:], rstd[:])
                    nc.vector.tensor_scalar(
                        out=xn_t[:, i * DIM : (i + 1) * DIM],
                        in0=xt,
                        scalar1=mv[:, 0:1],
                        scalar2=rstd[:],
                        op0=ALU.subtract,
                        op1=ALU.mult,
                    )
                # transpose to [dim, tokens], fusing gamma/beta; split the
                # scale/cast between Scalar and Vector engines
                for c in range(DC):
                    pt = ptr.tile([128, 512], MMDT, tag="tr")
                    for i in range(4):
                        nc.tensor.transpose(
                            pt[:, i * 128 : (i + 1) * 128],
                            xn_t[:, i * DIM + c * 128 : i * DIM + (c + 1) * 128],
                            ident[:],
                        )
                    dst = xnt[:, c * N + tg * 512 : c * N + (tg + 1) * 512]
                    # all on ACT: the Vector engine's LN stream is the wall
                    # that gates the exp start; ACT has ~16us of slack here
                    nc.scalar.activation(
                        dst,
                        pt[:],
                        AF.Identity,
                        bias=gbb_t[:, 8 + c : 9 + c],
                        scale=gbb_t[:, c : c + 1],
                    )

            def two_chains(width=512):
                c0 = pp.tile([128, 512], F32, tag="acc", name="c0")
                c1 = pkv.tile([128, 512], F32, tag="kvacc", name="c1")
                return [c0[:, 0:width], c1[:, 0:width]]

            def k_proj(s):
                pqs = two_chains()
                for c in range(DC):
                    for m in range(2):
                        nc.tensor.matmul(
                            pqs[m],
                            wk_sb[:, c * LIN + m * 128 : c * LIN + (m + 1) * 128],
                            xnt[:, c * N + s * 512 : c * N + (s + 1) * 512],
                            start=(c == 0),
                            stop=(c == DC - 1),
                        )
                for m in range(2):
                    for lh in range(2):
                        h = 2 * m + lh
                        span = slice(h * N + s * 512, h * N + (s + 1) * 512)
                        src = pqs[m][lh * 64 : lh * 64 + 64, :]
                        if (m + lh) % 2 == 0:
                            nc.vector.tensor_copy(kt_sb[0:64, span], src)
                        else:
                            nc.scalar.copy(kt_sb[0:64, span], src)
                        # K=128 duplication: SBUF->SBUF DMA, no HBM traffic
                        nc.sync.dma_start(kt_sb[64:128, span], kt_sb[0:64, span])

            def v_proj(s):
                for tp in range(2):
                    pvs = two_chains(width=LIN)
                    for c in range(DC):
                        for ti in range(2):
                            t = s * 4 + 2 * tp + ti
                            nc.tensor.matmul(
                                pvs[ti],
                                xnt[:, c * N + t * 128 : c * N + (t + 1) * 128],
                                wv_sb[:, c * LIN : (c + 1) * LIN],
                                start=(c == 0),
                                stop=(c == DC - 1),
                            )
                    for ti in range(2):
                        kc = s * 4 + 2 * tp + ti
                        vdst = vones[:, kc * VSTR + 64 :]
                        dst_ap = bass.AP(
                            tensor=vdst.tensor,
                            offset=vdst.offset,
                            ap=[vdst.ap[0], [192, 2], [1, 128]],
                        )
                        srcv = pvs[ti].rearrange("p (n f) -> p n f", f=128)
                        if ti == 0:
                            nc.vector.tensor_copy(dst_ap, srcv)
                        else:
                            nc.scalar.copy(dst_ap, srcv)

            def q_proj_m(m):
                # head pair m over all 4 spans: 4 interleaved chains
                c0 = pp.tile([128, 512], F32, tag="acc", name="q0")
                c1 = pp.tile([128, 512], F32, tag="acc", name="q1")
                c2 = pkv.tile([128, 512], F32, tag="kvacc", name="q2")
                c3 = pkv.tile([128, 512], F32, tag="kvacc", name="q3")
                pqs = [c0[:], c1[:], c2[:], c3[:]]
                for c in range(DC):
                    for s in range(TG):
                        nc.tensor.matmul(
                            pqs[s],
                            wq_sb[:, c * LIN + m * 128 : c * LIN + (m + 1) * 128],
                            xnt[:, c * N + s * 512 : c * N + (s + 1) * 512],
                            start=(c == 0),
                            stop=(c == DC - 1),
                        )
                for s in range(TG):
                    for lh in range(2):
                        h = 2 * m + lh
                        span = slice(h * N + s * 512, h * N + (s + 1) * 512)
                        src = pqs[s][lh * 64 : lh * 64 + 64, :]
                        if (s + lh) % 2 == 0:
                            nc.vector.tensor_copy(q_t[0:64, span], src)
                        else:
                            nc.scalar.copy(q_t[0:64, span], src)
                        nc.sync.dma_start(q_t[64:128, span], q_t[0:64, span])

            for tg in range(TG):
                ln_group(tg)
            for s in range(TG):
                k_proj(s)
            for s in range(TG):
                v_proj(s)
            def early_unit(h, d):
                # first-quarter unit emitted inside the proj scope, S^T in
                # [128, 512] singles from the pkv pool: the ACT exp stream
                # (the binding resource) starts ~40us before the remaining
                # projections drain
                po = pp.tile([128, 512], F32, tag="acc", name="po")
                for kc in range(KCH):
                    pss = pkv.tile([128, 512], F32, tag="kvacc", name="pse")
                    nc.tensor.matmul(
                        pss[:],
                        kt_sb[:, h * N + kc * 128 : h * N + (kc + 1) * 128],
                        q_t[:, h * N + d * 512 : h * N + (d + 1) * 512],
                        start=True,
                        stop=True,
                    )
                    if kc % 2 == 0:
                        es = es_pool.tile([128, 1024], MMDT, tag="es", name="es")
                        nc.scalar.activation(
                            es[:, 0:512], pss[:], AF.Exp, scale=SCALE / 2
                        )
                        esv = es[:, 0:512]
                    else:
                        e16 = es16_pool.tile(
                            [128, 1024], mybir.dt.int16, tag="e16", name="e16"
                        )
                        nc.vector.tensor_scalar(
                            out=e16[:, 0:512],
                            in0=pss[:],
                            scalar1=SCH_A,
                            scalar2=SCH_B,
                            op0=ALU.mult,
                            op1=ALU.add,
                        )
                        esv = e16[:, 0:512].bitcast(MMDT)
                    nc.tensor.matmul(
                        po[:],
                        av_lhs(h, kc),
                        esv,
                        start=(kc == 0),
                        stop=(kc == KCH - 1),
                    )
                head_divide(h, d, po)

            # Q heads 0-1, then the first four attention units (heads 0-1
            # of positions 0 and 1 need only Q chunk m0); their exp backlog
            # covers the Qm1 projection with no ACT gap
            q_proj_m(0)
            early_unit(0, 0)
            early_unit(1, 0)
            q_proj_m(1)

        # ---- attention + exchange ----------------------------------------
        ps_s = ctx.enter_context(
            tc.tile_pool(name="ps_s", bufs=3, space="PSUM")
        )
        rid = sp.partition_id() % GROUP

        # quarter-major; position d = absolute quarter (rid+1+d)%4 via the
        # host token rotation, so position 3 is my OWN quarter and needs no
        # exchange. Foreign quarters gather mid-attention.
        for d in range(GROUP):
            for h in range(LH):
                if d == 0 and h < 2:
                    continue  # emitted inside proj_ctx above
                head_unit(ps_s, h, d)
            if d < 3:
                stage_and_gather(d)
                # spread the W_out loads (2MB total) across the quarter
                # boundaries; a single 8-chunk burst was observed to stretch
                # concurrent ACT exp instructions
                emit_wout(range(2 + d * 2, 4 + d * 2))
        emit_wout([0, 1])
        # own tile -> relative slot 0
        nc.sync.dma_start(att_full[:, 0 : 2 * NQ], att_t[:, 6 * NQ : 8 * NQ])
        for d in range(3):
            extract(d, rid)

        # ---- output projection y^T = W_out^T @ att^T + b_out -------------
        # four interleaved accumulation chains; foreign att chunks (landed
        # mid-attention) accumulate first, own chunks last
        C_ORDER = [2, 3, 4, 5, 6, 7, 0, 1]
        for mp in range(DC // 4):
            pys = []
            for mi in range(4):
                if mi < 2:
                    py = pp.tile([128, 512], F32, tag="acc", name=f"py{mi}")
                    pys.append(py[:])
                else:
                    py = ps_s.tile([128, 1024], F32, tag="pss", name=f"py{mi}")
                    pys.append(py[:, 0:512])
            for ci, c in enumerate(C_ORDER):
                for mi in range(4):
                    m = 4 * mp + mi
                    nc.tensor.matmul(
                        pys[mi],
                        wout_sb[:, c * DIM + m * 128 : c * DIM + (m + 1) * 128],
                        att_full[:, c * NQ : (c + 1) * NQ],
                        start=(ci == 0),
                        stop=(ci == DC - 1),
                    )
            for mi in range(4):
                m = 4 * mp + mi
                y_sb = y_pool.tile([128, 512], MMDT, tag="y")
                nc.vector.tensor_scalar(
                    out=y_sb[:],
                    in0=pys[mi],
                    scalar1=gbb_t[:, 16 + m : 17 + m],
                    scalar2=None,
                    op0=ALU.add,
                )
                nc.sync.dma_start(out[m * 128 : (m + 1) * 128, :], y_sb[:])

    nc.compile()
    return nc


_NC_CACHE = None


def _get_nc():
    global _NC_CACHE
    if _NC_CACHE is None:
        _NC_CACHE = build_nc()
    return _NC_CACHE


def _make_in_maps(x, ln_gamma, ln_beta, W_qk, W_v, W_out, b_out):
    mmnp = mybir.dt.np(MMDT)
    wqk = np.asarray(W_qk, dtype=np.float32)
    wv = np.asarray(W_v, dtype=np.float32)
    wo = np.asarray(W_out, dtype=np.float32)
    gamma = np.asarray(ln_gamma, dtype=np.float32).reshape(DC, 128).T
    beta = np.asarray(ln_beta, dtype=np.float32).reshape(DC, 128).T
    bout = np.asarray(b_out, dtype=np.float32).reshape(DC, 128).T
    gbb = np.ascontiguousarray(np.concatenate([gamma, beta, bout], axis=1))
    xf = np.asarray(x, dtype=np.float32)
    xb = [np.ascontiguousarray(xf[g]).astype(mmnp) for g in range(B)]
    in_maps = []
    for i in range(NCORES):
        g, r = i // GROUP, i % GROUP
        cols = slice(r * LIN, (r + 1) * LIN)
        kcols = slice(INNER + r * LIN, INNER + (r + 1) * LIN)
        # token rotation: program position-quarter d = absolute quarter
        # (r+1+d)%4, so position 3 is this core's own quarter
        xr = np.roll(xb[g], -(r + 1) * NQ, axis=0)
        # W_out rows in relative-slot order: slot s holds the heads of
        # rank (r-s)%4 (slot 0 = own heads)
        wor = np.concatenate(
            [
                wo[((r - s) % GROUP) * LIN : (((r - s) % GROUP) + 1) * LIN]
                for s in range(GROUP)
            ]
        )
        in_maps.append(
            {
                "x": np.ascontiguousarray(xr),
                "gbb": gbb,
                "wqk_k": np.ascontiguousarray(wqk[:, kcols]).astype(mmnp),
                "wqk_q": np.ascontiguousarray(wqk[:, cols]).astype(mmnp),
                "W_v": np.ascontiguousarray(wv[:, cols]).astype(mmnp),
                "W_out": np.ascontiguousarray(wor).astype(mmnp),
            }
        )
    return in_maps


def run(inputs: dict, trace: bool = False):
    """Run the distributed kernel; returns (full_output, BassKernelResults)."""
    nc = _get_nc()
    in_maps = _make_in_maps(**inputs)
    res = run_bass_kernel_spmd(
        nc, in_maps, core_ids=list(range(NCORES)), trace=trace
    )
    out_full = np.empty((B, N, DIM), dtype=np.float32)
    for i in range(NCORES):
        g, r = i // GROUP, i % GROUP
        out_full[g, r * NQ : (r + 1) * NQ, :] = (
            res.results[i]["out"].astype(np.float32).T
        )
    return out_full, res


def kernel(**inputs) -> np.ndarray:
    out, _ = run(inputs, trace=False)
    return out
